# revision 14
# baseline (speedup 1.0000x reference)
"""Causal random-feature attention (sin/cos phi) Trainium2 kernel.

Sharding: data-parallel over batch B=8 -> one batch element per NeuronCore.
Each core runs an identical single-core program (no collectives).

Math notes (exact rewrites of the reference):
  - phi(u) = concat(sin(p), cos(p)) * K^-0.5 with p = (u * D^-0.25) @ rm^T.
    The K^-0.5 factor cancels in num/den except inside max(den, EPS); we drop
    it and clamp at EPS*K instead (max(c^2*d, e) = c^2*max(d, e/c^2)).
  - Wq/Wk are folded into the random matrices on the host (f64):
    A = W^T @ rm_scaled^T / 2pi, so the phase matmul directly yields phases
    in TURNS (p/2pi); bq/bk fold into per-feature offsets theta applied via
    the Sin activation bias (Sin computes sin(in*2pi + theta)).
  - Phase matmuls run in fp32r (PE rounds operands to 12-bit mantissa,
    ~2.4e-4 rel) at 4x the fp32 matmul rate. Phase abs error ~2e-3 rad.
  - Range reduction in turns: k = round(pt) via one dual-scalar op
    ((pt + MAGIC) - MAGIC, exact fp32 rounding trick), pr = pt - k in
    [-0.5, 0.5]. cos branch: ind = (pr >= 0.25), prc = pr - ind; the +pi/2
    enters via the sin bias (theta + pi/2 rows). All on DVE/Act - the slow
    GpSimd engine is never used.
  - cos(p) = sin(p + pi/2); the ACT Sin table is only accurate on ~[-3.2,
    3.2]; all sin args are within [-3pi/2 + pi/2, pi + pi/2 - 2pi*ind] ->
    [-pi, pi] (+theta, theta=0 here).
  - x arrives pre-transposed from the host (xT [E,T] f32 fed to fp32r
    tiles; x16 [E,T] f16 for the value path) - no on-device transposes of x.
  - bo is added on host; bv must be zero (asserted; true for this problem).
  - attn is scaled by 1/1024 into fp16 (den clamps make |attn| reach ~2e7);
    Wo^T is scaled by 1024 on the host. Out-projection runs in fp16.

Precision: the den-relevant path (scores, state, den) is kept at full fp32.
The value path (v, attn@Wo) uses fp16 - it enters the output linearly, a
~5e-4 relative effect. Phases carry fp32r noise (~2e-3 rad) into sin values.

Partition alignment: phases for a pair of heads (hA, hB) come out of one
accumulated matmul as PSUM rows 0:64 / 64:128. Compute engines cannot move
data across partitions, so the two "wrong half" slices (sin of hB, cos of
hA) are staged through one SBUF->SBUF DMA (the DMA fabric can shift
partitions); everything else is partition-aligned by construction.
"""

import math

import numpy as np

import concourse.bass as bass
import concourse.mybir as mybir
import concourse.tile as tile
from concourse import bacc
from concourse.bass_utils import run_bass_kernel_spmd
from concourse.masks import make_identity

T, B, E, H, D, K = 4096, 8, 1024, 8, 128, 64
K2 = 2 * K          # 128, phi dim
CHUNK = 128
MC = 512            # macro chunk (token rows)
NMC = T // MC       # 8
SC = MC // CHUNK    # 4 sub-chunks per macro chunk
NKE = E // 128      # 8 contraction tiles
EPS_K = 1e-6 * K    # eps with the K^-0.5 phi scaling folded in
TAU = 1.0
ATTN_SCALE = 1024.0  # attn carries 1/ATTN_SCALE in fp16; Wo^T carries ATTN_SCALE
MAGIC = 12582912.0   # 1.5 * 2^23: float32 round-to-nearest-int trick
TWOPI = float(2.0 * math.pi)

F32 = mybir.dt.float32
F32R = mybir.dt.float32r
F16 = mybir.dt.float16
BF16 = mybir.dt.bfloat16
Sin = mybir.ActivationFunctionType.Sin
Identity = mybir.ActivationFunctionType.Identity
Alu = mybir.AluOpType

_BUILT = None  # compiled program cache


def _build():
    nc = bacc.Bacc(None, target_bir_lowering=False)

    xh_d = nc.dram_tensor("xh", [E, T], BF16, kind="ExternalInput")
    xl_d = nc.dram_tensor("xl", [E, T], BF16, kind="ExternalInput")
    x16_d = nc.dram_tensor("x16", [E, T], F16, kind="ExternalInput")
    aqh_d = nc.dram_tensor("aqh", [E, H * K], BF16, kind="ExternalInput")
    aql_d = nc.dram_tensor("aql", [E, H * K], BF16, kind="ExternalInput")
    akh_d = nc.dram_tensor("akh", [E, H * K], BF16, kind="ExternalInput")
    akl_d = nc.dram_tensor("akl", [E, H * K], BF16, kind="ExternalInput")
    wvT_d = nc.dram_tensor("wvT", [E, E], F16, kind="ExternalInput")
    woT_d = nc.dram_tensor("woT", [E, E], F16, kind="ExternalInput")
    thq_d = nc.dram_tensor("thq", [128, H], F32, kind="ExternalInput")
    thk_d = nc.dram_tensor("thk", [128, H], F32, kind="ExternalInput")
    maskT_d = nc.dram_tensor("maskT", [CHUNK, CHUNK], F32, kind="ExternalInput")
    y_d = nc.dram_tensor("y", [T, E], F32, kind="ExternalOutput")

    with tile.TileContext(nc) as tc:
        with (
            tc.tile_pool(name="singles", bufs=1) as singles,
            tc.tile_pool(name="xyp", bufs=2) as xyp,
            tc.tile_pool(name="xtp", bufs=1) as xtp,
            tc.tile_pool(name="tgp", bufs=2) as tgp,
            tc.tile_pool(name="phip", bufs=1) as phip,
            tc.tile_pool(name="vp", bufs=1) as vp,
            tc.tile_pool(name="atp", bufs=2) as atp,
            tc.tile_pool(name="smal", bufs=8) as smal,
            tc.tile_pool(name="psa", bufs=2, space="PSUM") as psa,
            tc.tile_pool(name="psphi", bufs=2, space="PSUM") as psphi,
            tc.tile_pool(name="psb", bufs=2, space="PSUM") as psb,
            tc.tile_pool(name="psc", bufs=2, space="PSUM") as psc,
        ):
            # ---- persistent tiles ----
            aqh_sb = singles.tile([128, NKE, H * K], BF16, tag="aqh")
            aql_sb = singles.tile([128, NKE, H * K], BF16, tag="aql")
            akh_sb = singles.tile([128, NKE, H * K], BF16, tag="akh")
            akl_sb = singles.tile([128, NKE, H * K], BF16, tag="akl")
            wv_sb = singles.tile([128, NKE, E], F16, tag="wv")
            wo_sb = singles.tile([128, NKE, E], F16, tag="wo")
            thq_sb = singles.tile([128, H], F32, tag="thq")
            thk_sb = singles.tile([128, H], F32, tag="thk")
            mask_sb = singles.tile([128, 128], F32, tag="mask")
            id32 = singles.tile([128, 128], F32, tag="id32")
            id16 = singles.tile([128, 128], F16, tag="id16")
            s16 = singles.tile([128, H, 128], F16, tag="s16")
            z_sb = singles.tile([128, H], F32, tag="z")
            ones_sb = singles.tile([128, 1], F32, tag="ones")

            for dst_t, src_t in (
                (aqh_sb, aqh_d), (aql_sb, aql_d), (akh_sb, akh_d), (akl_sb, akl_d)
            ):
                nc.sync.dma_start(
                    out=dst_t, in_=src_t[:, :].rearrange("(ke p) m -> p ke m", p=128)
                )
            nc.sync.dma_start(
                out=wv_sb, in_=wvT_d[:, :].rearrange("(ke p) m -> p ke m", p=128)
            )
            nc.sync.dma_start(
                out=wo_sb, in_=woT_d[:, :].rearrange("(kh p) m -> p kh m", p=128)
            )
            nc.sync.dma_start(out=thq_sb, in_=thq_d[:, :])
            nc.sync.dma_start(out=thk_sb, in_=thk_d[:, :])
            nc.sync.dma_start(out=mask_sb, in_=maskT_d[:, :])
            make_identity(nc, id32)
            make_identity(nc, id16)
            nc.vector.memset(s16, 0.0)
            nc.vector.memset(z_sb, 0.0)
            nc.vector.memset(ones_sb, 1.0)

            def outproj(attnT, t0_row):
                for tt in range(SC):
                    y_t = xyp.tile([128, E], F32, tag="xy")
                    for eoh in range(2):
                        ps = psa.tile([128, 512], F32, tag="a")
                        for kh in range(H):
                            nc.tensor.matmul(
                                ps,
                                lhsT=attnT[:, kh, tt * 128 : (tt + 1) * 128],
                                rhs=wo_sb[:, kh, eoh * 512 : (eoh + 1) * 512],
                                start=(kh == 0),
                                stop=(kh == H - 1),
                            )
                        nc.vector.tensor_copy(y_t[:, eoh * 512 : (eoh + 1) * 512], ps)
                    nc.sync.dma_start(
                        out=y_d[t0_row + tt * 128 : t0_row + (tt + 1) * 128, :],
                        in_=y_t,
                    )

            prev = None
            for mc in range(NMC):
                t0_row = mc * MC

                # ---- load pre-transposed x tiles (bf16 hi/lo, f16) ----
                xh = xtp.tile([128, NKE, MC], BF16, tag="xh")
                xl = xtp.tile([128, NKE, MC], BF16, tag="xl")
                xT16 = xtp.tile([128, NKE, MC], F16, tag="xt16")
                for dst_t, src_t in ((xh, xh_d), (xl, xl_d), (xT16, x16_d)):
                    nc.sync.dma_start(
                        out=dst_t,
                        in_=src_t[:, t0_row : t0_row + MC].rearrange(
                            "(ke p) m -> p ke m", p=128
                        ),
                    )

                # ---- phases (turns) + sin/cos (paired heads) ----
                pq_sb = phip.tile([128, H * MC], F32, tag="pq")
                pk_sb = phip.tile([128, H * MC], F32, tag="pk")
                for ah_sb, al_sb, th_sb, dst in (
                    (aqh_sb, aql_sb, thq_sb, pq_sb),
                    (akh_sb, akl_sb, thk_sb, pk_sb),
                ):
                    for pg in range(H // 2):
                        hA, hB = 2 * pg, 2 * pg + 1
                        csl = slice(pg * 128, (pg + 1) * 128)
                        ps = psphi.tile([128, MC], F32, tag="phi")
                        chains = [(ah_sb, xh), (ah_sb, xl), (al_sb, xh)]
                        n_mm = len(chains) * NKE
                        i_mm = 0
                        for a_t, x_t in chains:
                            for ke in range(NKE):
                                nc.tensor.matmul(
                                    ps,
                                    lhsT=a_t[:, ke, csl],
                                    rhs=x_t[:, ke, :],
                                    start=(i_mm == 0),
                                    stop=(i_mm == n_mm - 1),
                                )
                                i_mm += 1
                        # k = round(pt) via (pt + MAGIC) - MAGIC (exact)
                        kk = tgp.tile([128, MC], F32, tag="kk")
                        nc.vector.tensor_scalar(
                            kk, ps, MAGIC, MAGIC,
                            op0=Alu.add, op1=Alu.subtract,
                        )
                        pr_s = tgp.tile([128, MC], F32, tag="prs")
                        nc.vector.scalar_tensor_tensor(
                            pr_s, kk, -1.0, ps,
                            op0=Alu.mult, op1=Alu.add,
                        )
                        # cos branch: ind = (pr_s >= 0.25); pr_c = pr_s - ind
                        pr_c = tgp.tile([128, MC], F32, tag="prc")
                        nc.vector.tensor_scalar(
                            pr_c, pr_s, 0.25, -1.0,
                            op0=Alu.is_ge, op1=Alu.mult,
                        )
                        nc.vector.tensor_add(pr_c, pr_c, pr_s)
                        # cross-half staging via SBUF->SBUF DMA (partition shift)
                        stage = tgp.tile([128, MC], F32, tag="stage")
                        nc.sync.dma_start(out=stage[0:64, :], in_=pr_s[64:128, :])
                        nc.sync.dma_start(out=stage[64:128, :], in_=pr_c[0:64, :])
                        # four aligned sin lookups: sin(pr*2pi + theta)
                        nc.scalar.activation(
                            dst[0:64, hA * MC : (hA + 1) * MC],
                            pr_s[0:64, :],
                            Sin,
                            scale=TWOPI,
                            bias=th_sb[0:64, hA : hA + 1],
                        )
                        nc.scalar.activation(
                            dst[64:128, hB * MC : (hB + 1) * MC],
                            pr_c[64:128, :],
                            Sin,
                            scale=TWOPI,
                            bias=th_sb[64:128, hB : hB + 1],
                        )
                        nc.scalar.activation(
                            dst[0:64, hB * MC : (hB + 1) * MC],
                            stage[0:64, :],
                            Sin,
                            scale=TWOPI,
                            bias=th_sb[0:64, hB : hB + 1],
                        )
                        nc.scalar.activation(
                            dst[64:128, hA * MC : (hA + 1) * MC],
                            stage[64:128, :],
                            Sin,
                            scale=TWOPI,
                            bias=th_sb[64:128, hA : hA + 1],
                        )

                if prev is not None:
                    outproj(*prev)

                # pq in fp16 for the value-path num matmuls
                pq16 = phip.tile([128, H * MC], F16, tag="pq16")
                nc.scalar.copy(pq16, pq_sb)

                # ---- v projection (fp16 in/out) ----
                v_sb = vp.tile([128, SC, H, 128], F16, tag="v")
                for tt in range(SC):
                    for eoh in range(2):
                        ps = psa.tile([128, 512], F32, tag="a")
                        for ke in range(NKE):
                            nc.tensor.matmul(
                                ps,
                                lhsT=xT16[:, ke, tt * 128 : (tt + 1) * 128],
                                rhs=wv_sb[:, ke, eoh * 512 : (eoh + 1) * 512],
                                start=(ke == 0),
                                stop=(ke == NKE - 1),
                            )
                        nc.vector.tensor_copy(
                            v_sb[:, tt, eoh * 4 : (eoh + 1) * 4, :],
                            ps[:, :].rearrange("p (a b) -> p a b", a=4),
                        )

                # ---- chunk recurrence (fp32) ----
                attnT = atp.tile([128, H, MC], F16, tag="attnT")
                for sc in range(SC):
                    sT_tiles = []
                    sT16_tiles = []
                    pkn_tiles = []
                    for h in range(H):
                        csl = slice(h * MC + sc * 128, h * MC + (sc + 1) * 128)
                        ps_s = psc.tile([128, 128], F32, tag="c")
                        nc.tensor.matmul(
                            ps_s,
                            lhsT=pk_sb[:, csl],
                            rhs=pq_sb[:, csl],
                            start=True,
                            stop=True,
                        )
                        sT = smal.tile([128, 128], F32, tag="sT")
                        nc.vector.tensor_mul(sT, ps_s, mask_sb)
                        sT_tiles.append(sT)
                        sT16 = smal.tile([128, 128], F16, tag="sT16")
                        nc.scalar.copy(sT16, sT)
                        sT16_tiles.append(sT16)

                        ps_p = psc.tile([128, 128], F32, tag="c")
                        nc.tensor.transpose(ps_p, pk_sb[:, csl], id32)
                        pkn = smal.tile([128, 128], F16, tag="pkn")
                        nc.scalar.copy(pkn, ps_p)
                        pkn_tiles.append(pkn)

                    a16_tiles = []
                    for h in range(H):
                        csl = slice(h * MC + sc * 128, h * MC + (sc + 1) * 128)
                        nd = psb.tile([128, 132], F32, tag="b")
                        # den (fp32, own psum group): colsum(masked scores)+pq.z
                        nc.tensor.matmul(
                            nd[:, 128:129],
                            lhsT=sT_tiles[h],
                            rhs=ones_sb,
                            start=True,
                            stop=False,
                        )
                        nc.tensor.matmul(
                            nd[:, 128:129],
                            lhsT=pq_sb[:, csl],
                            rhs=z_sb[:, h : h + 1],
                            start=False,
                            stop=True,
                        )
                        # value-path num (fp16): scores@v + pq@s
                        nc.tensor.matmul(
                            nd[:, 0:128],
                            lhsT=sT16_tiles[h],
                            rhs=v_sb[:, sc, h, :],
                            start=True,
                            stop=False,
                        )
                        nc.tensor.matmul(
                            nd[:, 0:128],
                            lhsT=pq16[:, csl],
                            rhs=s16[:, h, :],
                            start=False,
                            stop=True,
                        )
                        den = smal.tile([128, 1], F32, tag="den")
                        nc.vector.tensor_scalar(
                            den,
                            nd[:, 128:129],
                            EPS_K,
                            ATTN_SCALE,
                            op0=Alu.max,
                            op1=Alu.mult,
                        )
                        rec = smal.tile([128, 1], F32, tag="rec")
                        nc.vector.reciprocal(rec, den)
                        a16 = smal.tile([128, 128], F16, tag="a16")
                        nc.scalar.activation(
                            a16, nd[:, 0:128], Identity, scale=rec[:, 0:1]
                        )
                        a16_tiles.append(a16)

                        # state updates: z += colsum(pk) [fp32], s += pk^T@v [fp16]
                        zd = smal.tile([128, 1], F32, tag="zd")
                        nc.vector.tensor_reduce(
                            zd, pk_sb[:, csl], axis=mybir.AxisListType.X, op=Alu.add
                        )
                        nc.vector.tensor_add(
                            z_sb[:, h : h + 1], z_sb[:, h : h + 1], zd
                        )
                        delta = psb.tile([128, 132], F32, tag="b")
                        nc.tensor.matmul(
                            delta[:, 0:128],
                            lhsT=pkn_tiles[h],
                            rhs=v_sb[:, sc, h, :],
                            start=True,
                            stop=True,
                        )
                        nc.vector.tensor_add(
                            s16[:, h, :], s16[:, h, :], delta[:, 0:128]
                        )

                    for g in range(2):  # transpose attn tiles, 4 heads per batch
                        ps = psa.tile([128, 512], F16, tag="a")
                        for j in range(4):
                            nc.tensor.transpose(
                                ps[:, j * 128 : (j + 1) * 128],
                                a16_tiles[g * 4 + j],
                                id16,
                            )
                        nc.vector.tensor_copy(
                            attnT[:, g * 4 : (g + 1) * 4, sc * 128 : (sc + 1) * 128],
                            ps[:, :].rearrange("p (a b) -> p a b", a=4),
                        )

                prev = (attnT, t0_row)

            outproj(*prev)

    nc.compile()
    return nc


def _get_built():
    global _BUILT
    if _BUILT is None:
        _BUILT = _build()
    return _BUILT


def _prep_host(random_matrices, Wq, bq, Wk, bk, Wv, Wo, sigma):
    rm_scaled = (
        (sigma.astype(np.float64) * random_matrices.astype(np.float64))
        / TAU
        * (D ** -0.25)
    )  # [H, K, D]

    import ml_dtypes

    def fold(W, b):
        A = np.zeros((E, H * K), np.float64)
        th = np.zeros((128, H), np.float64)
        for h in range(H):
            blk = W.astype(np.float64)[h * 128 : (h + 1) * 128, :]  # [D, E]
            # phases in turns: fold 1/2pi into A
            A[:, h * K : (h + 1) * K] = (blk.T @ rm_scaled[h].T) / (2.0 * math.pi)
            tb = rm_scaled[h] @ b.astype(np.float64)[h * 128 : (h + 1) * 128]  # [K]
            th[0:64, h] = tb
            th[64:128, h] = tb + math.pi / 2
        # bf16 hi/lo split: A ~ Ah + Al to ~2^-17 relative
        Ah = A.astype(ml_dtypes.bfloat16)
        Al = (A - Ah.astype(np.float64)).astype(ml_dtypes.bfloat16)
        return Ah, Al, th.astype(np.float32)

    aqh, aql, thq = fold(Wq, bq)
    akh, akl, thk = fold(Wk, bk)
    wvT = np.ascontiguousarray(Wv.T).astype(np.float16)
    woT = np.ascontiguousarray(Wo.T * ATTN_SCALE).astype(np.float16)
    maskT = np.triu(np.ones((CHUNK, CHUNK), np.float32))
    return {
        "aqh": aqh,
        "aql": aql,
        "akh": akh,
        "akl": akl,
        "thq": thq,
        "thk": thk,
        "wvT": wvT,
        "woT": woT,
        "maskT": maskT,
    }


def kernel(
    x,
    random_matrices,
    Wq,
    bq,
    Wk,
    bk,
    Wv,
    bv,
    Wo,
    bo,
    sigma,
    _trace=False,
    _tmpdir=None,
):
    x = np.asarray(x, dtype=np.float32)
    args = [
        np.asarray(a, dtype=np.float32)
        for a in (random_matrices, Wq, bq, Wk, bk, Wv, bv, Wo, bo, sigma)
    ]
    random_matrices, Wq, bq, Wk, bk, Wv, bv, Wo, bo, sigma = args

    assert np.all(bv == 0.0), "kernel assumes bv == 0 (guaranteed by the problem)"

    import ml_dtypes

    shared = _prep_host(random_matrices, Wq, bq, Wk, bk, Wv, Wo, sigma)
    in_maps = []
    for b in range(B):
        m = dict(shared)
        xT = np.ascontiguousarray(x[:, b, :].T)  # [E, T]
        xh = xT.astype(ml_dtypes.bfloat16)
        m["xh"] = xh
        m["xl"] = (xT - xh.astype(np.float32)).astype(ml_dtypes.bfloat16)
        m["x16"] = xT.astype(np.float16)
        in_maps.append(m)

    nc = _get_built()
    res = run_bass_kernel_spmd(
        nc, in_maps, core_ids=list(range(B)), trace=_trace, tmpdir=_tmpdir
    )

    out = np.empty((T, B, E), dtype=np.float32)
    for b in range(B):
        out[:, b, :] = res.results[b]["y"] + bo[None, :]
    if _trace:
        return out, res
    return out


# revision 15
# speedup vs baseline: 1.0253x; 1.0253x over previous
"""Causal random-feature attention (sin/cos phi) Trainium2 kernel.

Sharding: data-parallel over batch B=8 -> one batch element per NeuronCore.
Each core runs an identical single-core program (no collectives).

Math notes (exact rewrites of the reference):
  - phi(u) = concat(sin(p), cos(p)) * K^-0.5 with p = (u * D^-0.25) @ rm^T.
    The K^-0.5 factor cancels in num/den except inside max(den, EPS); we drop
    it and clamp at EPS*K instead (max(c^2*d, e) = c^2*max(d, e/c^2)).
  - Wq/Wk are folded into the random matrices on the host (f64):
    A = W^T @ rm_scaled^T / 2pi, so the phase matmul directly yields phases
    in TURNS (p/2pi); bq/bk fold into per-feature offsets theta applied via
    the Sin activation bias (Sin computes sin(in*2pi + theta)).
  - Phase matmuls run in fp32r (PE rounds operands to 12-bit mantissa,
    ~2.4e-4 rel) at 4x the fp32 matmul rate. Phase abs error ~2e-3 rad.
  - Range reduction in turns: k = round(pt) via one dual-scalar op
    ((pt + MAGIC) - MAGIC, exact fp32 rounding trick), pr = pt - k in
    [-0.5, 0.5]. cos branch: ind = (pr >= 0.25), prc = pr - ind; the +pi/2
    enters via the sin bias (theta + pi/2 rows). All on DVE/Act - the slow
    GpSimd engine is never used.
  - cos(p) = sin(p + pi/2); the ACT Sin table is only accurate on ~[-3.2,
    3.2]; all sin args are within [-3pi/2 + pi/2, pi + pi/2 - 2pi*ind] ->
    [-pi, pi] (+theta, theta=0 here).
  - x arrives pre-transposed from the host (xT [E,T] f32 fed to fp32r
    tiles; x16 [E,T] f16 for the value path) - no on-device transposes of x.
  - bo is added on host; bv must be zero (asserted; true for this problem).
  - attn is scaled by 1/1024 into fp16 (den clamps make |attn| reach ~2e7);
    Wo^T is scaled by 1024 on the host. Out-projection runs in fp16.

Precision: the den-relevant path (scores, state, den) is kept at full fp32.
The value path (v, attn@Wo) uses fp16 - it enters the output linearly, a
~5e-4 relative effect. Phases carry fp32r noise (~2e-3 rad) into sin values.

Partition alignment: phases for a pair of heads (hA, hB) come out of one
accumulated matmul as PSUM rows 0:64 / 64:128. Compute engines cannot move
data across partitions, so the two "wrong half" slices (sin of hB, cos of
hA) are staged through one SBUF->SBUF DMA (the DMA fabric can shift
partitions); everything else is partition-aligned by construction.
"""

import math

import numpy as np

import concourse.bass as bass
import concourse.mybir as mybir
import concourse.tile as tile
from concourse import bacc
from concourse.bass_utils import run_bass_kernel_spmd
from concourse.masks import make_identity

T, B, E, H, D, K = 4096, 8, 1024, 8, 128, 64
K2 = 2 * K          # 128, phi dim
CHUNK = 128
MC = 512            # macro chunk (token rows)
NMC = T // MC       # 8
SC = MC // CHUNK    # 4 sub-chunks per macro chunk
NKE = E // 128      # 8 contraction tiles
EPS_K = 1e-6 * K    # eps with the K^-0.5 phi scaling folded in
TAU = 1.0
ATTN_SCALE = 1024.0  # attn carries 1/ATTN_SCALE in fp16; Wo^T carries ATTN_SCALE
MAGIC = 12582912.0   # 1.5 * 2^23: float32 round-to-nearest-int trick
TWOPI = float(2.0 * math.pi)

F32 = mybir.dt.float32
F32R = mybir.dt.float32r
F16 = mybir.dt.float16
BF16 = mybir.dt.bfloat16
Sin = mybir.ActivationFunctionType.Sin
Identity = mybir.ActivationFunctionType.Identity
Alu = mybir.AluOpType

_BUILT = None  # compiled program cache


def _build():
    nc = bacc.Bacc(None, target_bir_lowering=False)

    xh_d = nc.dram_tensor("xh", [E, T], BF16, kind="ExternalInput")
    xl_d = nc.dram_tensor("xl", [E, T], BF16, kind="ExternalInput")
    x16_d = nc.dram_tensor("x16", [E, T], F16, kind="ExternalInput")
    aqh_d = nc.dram_tensor("aqh", [E, H * K], BF16, kind="ExternalInput")
    aql_d = nc.dram_tensor("aql", [E, H * K], BF16, kind="ExternalInput")
    akh_d = nc.dram_tensor("akh", [E, H * K], BF16, kind="ExternalInput")
    akl_d = nc.dram_tensor("akl", [E, H * K], BF16, kind="ExternalInput")
    wvT_d = nc.dram_tensor("wvT", [E, E], F16, kind="ExternalInput")
    woT_d = nc.dram_tensor("woT", [E, E], F16, kind="ExternalInput")
    thq_d = nc.dram_tensor("thq", [128, H], F32, kind="ExternalInput")
    thk_d = nc.dram_tensor("thk", [128, H], F32, kind="ExternalInput")
    maskT_d = nc.dram_tensor("maskT", [CHUNK, CHUNK], F32, kind="ExternalInput")
    y_d = nc.dram_tensor("y", [T, E], F32, kind="ExternalOutput")

    with tile.TileContext(nc) as tc:
        with (
            tc.tile_pool(name="singles", bufs=1) as singles,
            tc.tile_pool(name="xyp", bufs=2) as xyp,
            tc.tile_pool(name="xtp", bufs=2) as xtp,
            tc.tile_pool(name="tgp", bufs=2) as tgp,
            tc.tile_pool(name="phip", bufs=1) as phip,
            tc.tile_pool(name="vp", bufs=1) as vp,
            tc.tile_pool(name="atp", bufs=1) as atp,
            tc.tile_pool(name="smal", bufs=8) as smal,
            tc.tile_pool(name="psa", bufs=2, space="PSUM") as psa,
            tc.tile_pool(name="psphi", bufs=2, space="PSUM") as psphi,
            tc.tile_pool(name="psb", bufs=2, space="PSUM") as psb,
            tc.tile_pool(name="psc", bufs=2, space="PSUM") as psc,
        ):
            # ---- persistent tiles ----
            aqh_sb = singles.tile([128, NKE, H * K], BF16, tag="aqh")
            aql_sb = singles.tile([128, NKE, H * K], BF16, tag="aql")
            akh_sb = singles.tile([128, NKE, H * K], BF16, tag="akh")
            akl_sb = singles.tile([128, NKE, H * K], BF16, tag="akl")
            wv_sb = singles.tile([128, NKE, E], F16, tag="wv")
            wo_sb = singles.tile([128, NKE, E], F16, tag="wo")
            thq_sb = singles.tile([128, H], F32, tag="thq")
            thk_sb = singles.tile([128, H], F32, tag="thk")
            mask_sb = singles.tile([128, 128], F32, tag="mask")
            id32 = singles.tile([128, 128], F32, tag="id32")
            id16 = singles.tile([128, 128], F16, tag="id16")
            s16 = singles.tile([128, H, 128], F16, tag="s16")
            z_sb = singles.tile([128, H], F32, tag="z")
            ones_sb = singles.tile([128, 1], F32, tag="ones")

            for dst_t, src_t in (
                (aqh_sb, aqh_d), (aql_sb, aql_d), (akh_sb, akh_d), (akl_sb, akl_d)
            ):
                nc.sync.dma_start(
                    out=dst_t, in_=src_t[:, :].rearrange("(ke p) m -> p ke m", p=128)
                )
            nc.sync.dma_start(out=thq_sb, in_=thq_d[:, :])
            nc.sync.dma_start(out=thk_sb, in_=thk_d[:, :])
            nc.sync.dma_start(out=mask_sb, in_=maskT_d[:, :])
            make_identity(nc, id32)
            make_identity(nc, id16)
            nc.vector.memset(s16, 0.0)
            nc.vector.memset(z_sb, 0.0)
            nc.vector.memset(ones_sb, 1.0)

            def outproj(attnT, t0_row):
                pass
            def _outproj_real(attnT, t0_row):
                for tt in range(SC):
                    y_t = xyp.tile([128, E], F32, tag="xy")
                    for eoh in range(2):
                        ps = psa.tile([128, 512], F32, tag="a")
                        for kh in range(H):
                            nc.tensor.matmul(
                                ps,
                                lhsT=attnT[:, kh, tt * 128 : (tt + 1) * 128],
                                rhs=wo_sb[:, kh, eoh * 512 : (eoh + 1) * 512],
                                start=(kh == 0),
                                stop=(kh == H - 1),
                            )
                        nc.vector.tensor_copy(y_t[:, eoh * 512 : (eoh + 1) * 512], ps)
                    nc.sync.dma_start(
                        out=y_d[t0_row + tt * 128 : t0_row + (tt + 1) * 128, :],
                        in_=y_t,
                    )

            for mc in range(NMC):
                t0_row = mc * MC

                # ---- load pre-transposed x tiles (bf16 hi/lo, f16) ----
                xh = xtp.tile([128, NKE, MC], BF16, tag="xh")
                xl = xtp.tile([128, NKE, MC], BF16, tag="xl")
                xT16 = xtp.tile([128, NKE, MC], F16, tag="xt16")
                for dst_t, src_t in ((xh, xh_d), (xl, xl_d), (xT16, x16_d)):
                    nc.sync.dma_start(
                        out=dst_t,
                        in_=src_t[:, t0_row : t0_row + MC].rearrange(
                            "(ke p) m -> p ke m", p=128
                        ),
                    )

                if mc == 0:
                    # weight DMAs issued after mc=0 x tiles so phases start ASAP
                    nc.sync.dma_start(
                        out=wv_sb,
                        in_=wvT_d[:, :].rearrange("(ke p) m -> p ke m", p=128),
                    )
                    nc.sync.dma_start(
                        out=wo_sb,
                        in_=woT_d[:, :].rearrange("(kh p) m -> p kh m", p=128),
                    )

                # ---- phases (turns) + sin/cos (paired heads) ----
                pq_sb = phip.tile([128, H * MC], F32, tag="pq")
                pk_sb = phip.tile([128, H * MC], F32, tag="pk")
                for ah_sb, al_sb, th_sb, dst in (
                    (aqh_sb, aql_sb, thq_sb, pq_sb),
                    (akh_sb, akl_sb, thk_sb, pk_sb),
                ):
                    for pg in range(H // 2):
                        hA, hB = 2 * pg, 2 * pg + 1
                        csl = slice(pg * 128, (pg + 1) * 128)
                        ps = psphi.tile([128, MC], F32, tag="phi")
                        chains = [(ah_sb, xh), (ah_sb, xl), (al_sb, xh)]
                        n_mm = len(chains) * NKE
                        i_mm = 0
                        for a_t, x_t in chains:
                            for ke in range(NKE):
                                nc.tensor.matmul(
                                    ps,
                                    lhsT=a_t[:, ke, csl],
                                    rhs=x_t[:, ke, :],
                                    start=(i_mm == 0),
                                    stop=(i_mm == n_mm - 1),
                                )
                                i_mm += 1
                        # k = round(pt) via (pt + MAGIC) - MAGIC (exact)
                        kk = tgp.tile([128, MC], F32, tag="kk")
                        nc.vector.tensor_scalar(
                            kk, ps, MAGIC, MAGIC,
                            op0=Alu.add, op1=Alu.subtract,
                        )
                        pr_s = tgp.tile([128, MC], F32, tag="prs")
                        nc.vector.scalar_tensor_tensor(
                            pr_s, kk, -1.0, ps,
                            op0=Alu.mult, op1=Alu.add,
                        )
                        # cos branch: ind = (pr_s >= 0.25); pr_c = pr_s - ind
                        pr_c = tgp.tile([128, MC], F32, tag="prc")
                        nc.vector.tensor_scalar(
                            pr_c, pr_s, 0.25, -1.0,
                            op0=Alu.is_ge, op1=Alu.mult,
                        )
                        nc.vector.tensor_add(pr_c, pr_c, pr_s)
                        # cross-half staging via SBUF->SBUF DMA (partition shift)
                        stage = tgp.tile([128, MC], F32, tag="stage")
                        nc.sync.dma_start(out=stage[0:64, :], in_=pr_s[64:128, :])
                        nc.sync.dma_start(out=stage[64:128, :], in_=pr_c[0:64, :])
                        # four aligned sin lookups: sin(pr*2pi + theta)
                        nc.scalar.activation(
                            dst[0:64, hA * MC : (hA + 1) * MC],
                            pr_s[0:64, :],
                            Sin,
                            scale=TWOPI,
                            bias=th_sb[0:64, hA : hA + 1],
                        )
                        nc.scalar.activation(
                            dst[64:128, hB * MC : (hB + 1) * MC],
                            pr_c[64:128, :],
                            Sin,
                            scale=TWOPI,
                            bias=th_sb[64:128, hB : hB + 1],
                        )
                        nc.scalar.activation(
                            dst[0:64, hB * MC : (hB + 1) * MC],
                            stage[0:64, :],
                            Sin,
                            scale=TWOPI,
                            bias=th_sb[0:64, hB : hB + 1],
                        )
                        nc.scalar.activation(
                            dst[64:128, hA * MC : (hA + 1) * MC],
                            stage[64:128, :],
                            Sin,
                            scale=TWOPI,
                            bias=th_sb[64:128, hA : hA + 1],
                        )

                # pq in fp16 for the value-path num matmuls
                pq16 = phip.tile([128, H * MC], F16, tag="pq16")
                nc.scalar.copy(pq16, pq_sb)

                # ---- v projection (fp16 in/out) ----
                v_sb = vp.tile([128, SC, H, 128], F16, tag="v")
                for tt in range(SC):
                    for eoh in range(2):
                        ps = psa.tile([128, 512], F32, tag="a")
                        for ke in range(NKE):
                            nc.tensor.matmul(
                                ps,
                                lhsT=xT16[:, ke, tt * 128 : (tt + 1) * 128],
                                rhs=wv_sb[:, ke, eoh * 512 : (eoh + 1) * 512],
                                start=(ke == 0),
                                stop=(ke == NKE - 1),
                            )
                        nc.vector.tensor_copy(
                            v_sb[:, tt, eoh * 4 : (eoh + 1) * 4, :],
                            ps[:, :].rearrange("p (a b) -> p a b", a=4),
                        )

                # ---- chunk recurrence (fp32) ----
                attnT = atp.tile([128, H, MC], F16, tag="attnT")
                for sc in range(SC):
                    sT_tiles = []
                    sT16_tiles = []
                    pkn_tiles = []
                    for h in range(H):
                        csl = slice(h * MC + sc * 128, h * MC + (sc + 1) * 128)
                        ps_s = psc.tile([128, 128], F32, tag="c")
                        nc.tensor.matmul(
                            ps_s,
                            lhsT=pk_sb[:, csl],
                            rhs=pq_sb[:, csl],
                            start=True,
                            stop=True,
                        )
                        sT = smal.tile([128, 128], F32, tag="sT")
                        nc.vector.tensor_mul(sT, ps_s, mask_sb)
                        sT_tiles.append(sT)
                        sT16 = smal.tile([128, 128], F16, tag="sT16")
                        nc.scalar.copy(sT16, sT)
                        sT16_tiles.append(sT16)

                        ps_p = psc.tile([128, 128], F32, tag="c")
                        nc.tensor.transpose(ps_p, pk_sb[:, csl], id32)
                        pkn = smal.tile([128, 128], F16, tag="pkn")
                        nc.scalar.copy(pkn, ps_p)
                        pkn_tiles.append(pkn)

                    a16_tiles = []
                    for h in range(H):
                        csl = slice(h * MC + sc * 128, h * MC + (sc + 1) * 128)
                        nd = psb.tile([128, 132], F32, tag="b")
                        # den (fp32, own psum group): colsum(masked scores)+pq.z
                        nc.tensor.matmul(
                            nd[:, 128:129],
                            lhsT=sT_tiles[h],
                            rhs=ones_sb,
                            start=True,
                            stop=False,
                        )
                        nc.tensor.matmul(
                            nd[:, 128:129],
                            lhsT=pq_sb[:, csl],
                            rhs=z_sb[:, h : h + 1],
                            start=False,
                            stop=True,
                        )
                        # value-path num (fp16): scores@v + pq@s
                        nc.tensor.matmul(
                            nd[:, 0:128],
                            lhsT=sT16_tiles[h],
                            rhs=v_sb[:, sc, h, :],
                            start=True,
                            stop=False,
                        )
                        nc.tensor.matmul(
                            nd[:, 0:128],
                            lhsT=pq16[:, csl],
                            rhs=s16[:, h, :],
                            start=False,
                            stop=True,
                        )
                        den = smal.tile([128, 1], F32, tag="den")
                        nc.vector.tensor_scalar(
                            den,
                            nd[:, 128:129],
                            EPS_K,
                            ATTN_SCALE,
                            op0=Alu.max,
                            op1=Alu.mult,
                        )
                        rec = smal.tile([128, 1], F32, tag="rec")
                        nc.vector.reciprocal(rec, den)
                        a16 = smal.tile([128, 128], F16, tag="a16")
                        nc.scalar.activation(
                            a16, nd[:, 0:128], Identity, scale=rec[:, 0:1]
                        )
                        a16_tiles.append(a16)

                        # state updates: z += colsum(pk) [fp32], s += pk^T@v [fp16]
                        zd = smal.tile([128, 1], F32, tag="zd")
                        nc.vector.tensor_reduce(
                            zd, pk_sb[:, csl], axis=mybir.AxisListType.X, op=Alu.add
                        )
                        nc.vector.tensor_add(
                            z_sb[:, h : h + 1], z_sb[:, h : h + 1], zd
                        )
                        delta = psb.tile([128, 132], F32, tag="b")
                        nc.tensor.matmul(
                            delta[:, 0:128],
                            lhsT=pkn_tiles[h],
                            rhs=v_sb[:, sc, h, :],
                            start=True,
                            stop=True,
                        )
                        nc.vector.tensor_add(
                            s16[:, h, :], s16[:, h, :], delta[:, 0:128]
                        )

                    for g in range(2):  # transpose attn tiles, 4 heads per batch
                        ps = psa.tile([128, 512], F16, tag="a")
                        for j in range(4):
                            nc.tensor.transpose(
                                ps[:, j * 128 : (j + 1) * 128],
                                a16_tiles[g * 4 + j],
                                id16,
                            )
                        nc.vector.tensor_copy(
                            attnT[:, g * 4 : (g + 1) * 4, sc * 128 : (sc + 1) * 128],
                            ps[:, :].rearrange("p (a b) -> p a b", a=4),
                        )

                _outproj_real(attnT, t0_row)

    nc.compile()
    return nc


def _get_built():
    global _BUILT
    if _BUILT is None:
        _BUILT = _build()
    return _BUILT


def _prep_host(random_matrices, Wq, bq, Wk, bk, Wv, Wo, sigma):
    rm_scaled = (
        (sigma.astype(np.float64) * random_matrices.astype(np.float64))
        / TAU
        * (D ** -0.25)
    )  # [H, K, D]

    import ml_dtypes

    def fold(W, b):
        A = np.zeros((E, H * K), np.float64)
        th = np.zeros((128, H), np.float64)
        for h in range(H):
            blk = W.astype(np.float64)[h * 128 : (h + 1) * 128, :]  # [D, E]
            # phases in turns: fold 1/2pi into A
            A[:, h * K : (h + 1) * K] = (blk.T @ rm_scaled[h].T) / (2.0 * math.pi)
            tb = rm_scaled[h] @ b.astype(np.float64)[h * 128 : (h + 1) * 128]  # [K]
            th[0:64, h] = tb
            th[64:128, h] = tb + math.pi / 2
        # bf16 hi/lo split: A ~ Ah + Al to ~2^-17 relative
        Ah = A.astype(ml_dtypes.bfloat16)
        Al = (A - Ah.astype(np.float64)).astype(ml_dtypes.bfloat16)
        return Ah, Al, th.astype(np.float32)

    aqh, aql, thq = fold(Wq, bq)
    akh, akl, thk = fold(Wk, bk)
    wvT = np.ascontiguousarray(Wv.T).astype(np.float16)
    woT = np.ascontiguousarray(Wo.T * ATTN_SCALE).astype(np.float16)
    maskT = np.triu(np.ones((CHUNK, CHUNK), np.float32))
    return {
        "aqh": aqh,
        "aql": aql,
        "akh": akh,
        "akl": akl,
        "thq": thq,
        "thk": thk,
        "wvT": wvT,
        "woT": woT,
        "maskT": maskT,
    }


def kernel(
    x,
    random_matrices,
    Wq,
    bq,
    Wk,
    bk,
    Wv,
    bv,
    Wo,
    bo,
    sigma,
    _trace=False,
    _tmpdir=None,
):
    x = np.asarray(x, dtype=np.float32)
    args = [
        np.asarray(a, dtype=np.float32)
        for a in (random_matrices, Wq, bq, Wk, bk, Wv, bv, Wo, bo, sigma)
    ]
    random_matrices, Wq, bq, Wk, bk, Wv, bv, Wo, bo, sigma = args

    assert np.all(bv == 0.0), "kernel assumes bv == 0 (guaranteed by the problem)"

    import ml_dtypes

    shared = _prep_host(random_matrices, Wq, bq, Wk, bk, Wv, Wo, sigma)
    in_maps = []
    for b in range(B):
        m = dict(shared)
        xT = np.ascontiguousarray(x[:, b, :].T)  # [E, T]
        xh = xT.astype(ml_dtypes.bfloat16)
        m["xh"] = xh
        m["xl"] = (xT - xh.astype(np.float32)).astype(ml_dtypes.bfloat16)
        m["x16"] = xT.astype(np.float16)
        in_maps.append(m)

    nc = _get_built()
    res = run_bass_kernel_spmd(
        nc, in_maps, core_ids=list(range(B)), trace=_trace, tmpdir=_tmpdir
    )

    out = np.empty((T, B, E), dtype=np.float32)
    for b in range(B):
        out[:, b, :] = res.results[b]["y"] + bo[None, :]
    if _trace:
        return out, res
    return out
